# revision 16
# baseline (speedup 1.0000x reference)
"""DiagonalLinear on 8 TRN2 NeuronCores.

y = x * clip(diagonal, -0.95, 0.95)  with x [16384, 8192] f32, diagonal [8192] f32.

Data-parallel: x is sharded along the batch dim (2048 rows per core), the
diagonal is replicated. Per core: one 0-stride DMA replicates the diagonal
across the 128 SBUF partitions, one DVE op clamps it, then 16 tiles of
[128, 8192] f32 (4 MiB contiguous DMAs) stream through a load -> DVE mul ->
store pipeline. Loads issue on the SP HWDGE ring, stores on the ACT HWDGE
ring, so the two streams overlap. Purely memory-bound.

Raw Bass (no TileContext): this walrus build rejects Tile's multi-wait
kernel-tail drain, and manual sync keeps every instruction at <=1 sem wait.
The kernel ends with barrier -> sem reset -> barrier so the NEFF is safely
re-executable (NTFF profiling reruns it with leftover sem values otherwise).
"""

import numpy as np

import concourse.bass as bass
import concourse.mybir as mybir
from concourse.bass_utils import run_bass_kernel_spmd

BATCH = 16384
LATENT = 8192
N_CORES = 8
ROWS_PER_CORE = BATCH // N_CORES  # 2048
P = 128
N_TILES = ROWS_PER_CORE // P  # 16
NBUF = 4

_NC_CACHE: dict[str, bass.Bass] = {}


def _build() -> bass.Bass:
    if "nc" in _NC_CACHE:
        return _NC_CACHE["nc"]

    nc = bass.Bass()
    x = nc.dram_tensor(
        "x", [ROWS_PER_CORE, LATENT], mybir.dt.float32, kind="ExternalInput"
    )
    # diagonal arrives pre-replicated across the 128 partitions (host-side
    # marshalling, same as sharding x) so its load is a normal parallel HBM
    # read instead of 128 serialized reads of one 32 KiB region.
    d = nc.dram_tensor(
        "diagonal", [P, LATENT], mybir.dt.float32, kind="ExternalInput"
    )
    out = nc.dram_tensor(
        "out", [ROWS_PER_CORE, LATENT], mybir.dt.float32, kind="ExternalOutput"
    )

    xt = x.rearrange("(n p) m -> n p m", p=P)  # [16, 128, 8192]
    ot = out.rearrange("(n p) m -> n p m", p=P)

    def buf(i):
        b = i % NBUF
        return slice(b * LATENT, (b + 1) * LATENT)

    with (
        nc.sbuf_tensor([P, NBUF * LATENT], mybir.dt.float32) as xbuf,
        nc.sbuf_tensor([P, LATENT], mybir.dt.float32) as dbc,
        nc.semaphore("ls") as ls,  # load completions (+16 each)
        nc.semaphore("ms") as ms,  # mul-drained markers (+1 each)
        nc.semaphore("ss") as ss,  # store completions (+16 each)
        nc.semaphore("bs") as bs,  # diag broadcast DMA (+16)
    ):
        # --- SP engine: x tile loads ---
        for i in range(N_TILES):
            if i >= NBUF:
                # buffer reused: wait for store of tile i-NBUF to land
                nc.sync.wait_ge(ss, 16 * (i - NBUF + 1))
            nc.sync.dma_start(out=xbuf[:, buf(i)], in_=xt[i]).then_inc(ls, 16)

        # --- ACT engine: diag load + stores ---
        nc.scalar.dma_start(out=dbc[:], in_=d[:]).then_inc(bs, 16)
        for i in range(N_TILES):
            nc.scalar.wait_ge(ms, i + 1)
            nc.scalar.dma_start(out=ot[i], in_=xbuf[:, buf(i)]).then_inc(ss, 16)
        nc.scalar.wait_ge(ss, 16 * N_TILES)

        # --- DVE engine: clamp + muls ---
        nc.vector.wait_ge(bs, 16)
        # clamp(d, -0.95, 0.95) = min(max(d, -0.95), 0.95), one DVE op
        nc.vector.tensor_scalar(
            out=dbc[:],
            in0=dbc[:],
            scalar1=-0.95,
            scalar2=0.95,
            op0=mybir.AluOpType.max,
            op1=mybir.AluOpType.min,
        )
        for i in range(N_TILES):
            nc.vector.wait_ge(ls, 16 * (i + 1))
            nc.vector.tensor_mul(xbuf[:, buf(i)], xbuf[:, buf(i)], dbc[:])
            # The store-gating inc rides on a separate tiny DVE op: the DVE's
            # per-op DRAIN means this op can only issue after the mul's writes
            # have fully left the pipe, so the inc postdates write visibility.
            nc.vector.tensor_scalar_mul(dbc[:, 0:1], dbc[:, 0:1], 1.0).then_inc(ms, 1)

        # --- tail: reset sems so the NEFF is safely re-executable (NTFF
        # profiling reruns it; leftover sem values would void every wait).
        # Mirrors TileContext._drain_and_barrier: barrier -> reset -> barrier.
        nc.all_engine_barrier()
        for s in (ls, ms, ss, bs):
            nc.gpsimd.dma_reset(range(s.num, s.num + 1))
            nc.gpsimd.sem_clear(s)
        nc.all_engine_barrier()

    _NC_CACHE["nc"] = nc
    return nc


def run(x: np.ndarray, diagonal: np.ndarray, trace: bool = False, **trace_kw):
    """Returns (full_output, BassKernelResults)."""
    x = np.asarray(x, dtype=np.float32)
    diagonal = np.asarray(diagonal, dtype=np.float32)
    assert x.shape == (BATCH, LATENT) and diagonal.shape == (LATENT,)

    nc = _build()
    diag_rep = np.ascontiguousarray(np.broadcast_to(diagonal, (P, LATENT)))
    in_maps = [
        {
            "x": np.ascontiguousarray(x[c * ROWS_PER_CORE : (c + 1) * ROWS_PER_CORE]),
            "diagonal": diag_rep,
        }
        for c in range(N_CORES)
    ]
    res = run_bass_kernel_spmd(
        nc, in_maps, core_ids=list(range(N_CORES)), trace=trace, **trace_kw
    )
    full = np.concatenate([res.results[c]["out"] for c in range(N_CORES)], axis=0)
    return full, res


def kernel(x: np.ndarray, diagonal: np.ndarray) -> np.ndarray:
    full, _ = run(x, diagonal, trace=False)
    return full
